# revision 9
# baseline (speedup 1.0000x reference)
"""Trainium2 Bass kernel for nn_DMF_50130858279442.

Reference computation (M=100 Monte-Carlo mutations, fixed RNG key 42):
    std_h[b]   = std(f_h[b,:], ddof=1)                     (per half h)
    G_h        = mask * normal_draw            [M,B,D]     (input-INDEPENDENT)
    cand       = f_h[None] + G_h * std_h                   [M,B,D]
    var[b,d]   = var_m(cand) + eps  = std_h[b]^2 * var_m(G_h)[b,d] + eps
    w          = (1/var) / sum_b(1/var)
    Z[b,d]     = sum_m(cand*w) = w * M * (f_h[b,d] + std_h[b]*mean_m(G_h)[b,d])

Because the RNG key is fixed, A = mean_m(G) and V = var_m(G) are constants
([B,D] per half) precomputed once on host CPU. The device kernel computes the
closed form.

Sharding: over the 2D=1024 OUTPUT COLUMNS (128 per core) — cores 0-3 own the
spatial half's columns, 4-7 the spectral half's. The batch-axis normalization
sum is then core-local (no collective; an AllReduce here pays a ~47us
cross-core launch-skew barrier on this stack). Each core redundantly computes
the row-std of its own half from an fp16 copy (1 MiB DMA; stats error ~1e-4).

Per-core layout: the 1024 batch rows fold into KB=8 row blocks of P=128 (the
SBUF partition dim); block k sits at free offset k*W. Row b=128k+p is at
[partition p, block k]. The whole pipeline is per-block so everything hides
behind the serial ACT square backbone.

Numerics shortcuts (all ~<=1e-4 rel, vs typical 1e-2 tolerances):
  - fp16 f for row stats; fp16 A (enters as small correction s*A to f).
  - eps folded into the V constant: t = u*(V+eps)/(D-1) = var + eps*std^2.
  - reciprocal_approx_fast (~51 ULP) instead of DVE RECIPROCAL (5x faster).
"""

import numpy as np

P = 128          # SBUF partitions = rows per block
KB = 8           # row blocks (B = KB*P)
D = 512          # per-half feature dim
W = 128          # output columns per core
NCORES = 8
M_MUT = 100      # NUM_MUTATIONS
RATE = 0.2       # MUTATION_RATE
EPS = 1e-6
B = 1024

_CACHE = {}


def _gstats():
    """A = mean_m(mask*normal), Vp = (var_m(mask*normal)+eps)/(D-1), both halves.

    Computed once on host CPU with the reference's exact RNG (jax threefry,
    key 42). Vp folds both the 1/(D-1) of the unbiased row-variance and the
    reference's +eps, so the device computes var_cand+eps ~= u * Vp with
    u = sumsq - sum^2/D (= (D-1)*std^2).
    """
    if "gstats" in _CACHE:
        return _CACHE["gstats"]
    import jax
    import jax.numpy as jnp

    cpu = jax.devices("cpu")[0]
    out = {}
    with jax.default_device(cpu):
        rng = jax.random.key(42)
        km1, kn1, km2, kn2 = jax.random.split(rng, 4)
        for name, km, kn in (("sp", km1, kn1), ("spec", km2, kn2)):
            mask = np.asarray(
                jax.random.uniform(km, (M_MUT, B, D), dtype=jnp.float32) < RATE
            )
            noise = np.asarray(
                jax.random.normal(kn, (M_MUT, B, D), dtype=jnp.float32)
            )
            G = np.where(mask, noise, np.float32(0.0)).astype(np.float64)
            A = G.mean(axis=0)
            V = G.var(axis=0)  # ddof=0, matches jnp.var over mutations
            out[name] = (
                np.ascontiguousarray(A, dtype=np.float16),
                np.ascontiguousarray((V + EPS) / (D - 1), dtype=np.float32),
            )
    _CACHE["gstats"] = out
    return out


def _build_bass():
    """Per-core SPMD Tile kernel (identical program; per-core data differs)."""
    if "nc" in _CACHE:
        return _CACHE["nc"]
    import concourse.bacc as bacc
    import concourse.mybir as mybir
    from concourse import bass
    from concourse import tile

    f32 = mybir.dt.float32
    f16 = mybir.dt.float16
    AF = mybir.ActivationFunctionType
    OP = mybir.AluOpType

    nc = bacc.Bacc(
        "TRN2", target_bir_lowering=False, debug=False, num_devices=NCORES
    )

    # the core's FULL half in fp16, block-major [KB, P, D] (row stats only)
    f_blk = nc.dram_tensor("f_blk", [KB, P, D], f16, kind="ExternalInput")
    # column slices for this core's W output columns, block-concat [P, KB*W]
    fc_d = nc.dram_tensor("fc", [P, KB * W], f32, kind="ExternalInput")
    a_d = nc.dram_tensor("ac", [P, KB * W], f16, kind="ExternalInput")
    v_d = nc.dram_tensor("vc", [P, KB * W], f32, kind="ExternalInput")
    z_d = nc.dram_tensor("z", [P, KB * W], f32, kind="ExternalOutput")

    def bcast_w(ap, blocks=KB):
        # free-axis broadcast: [P, W] -> [P, (blocks), W] with step-0 mid dim
        return bass.AP(ap.tensor, ap.offset, [ap.ap[0], [0, blocks], ap.ap[1]])

    with tile.TileContext(nc) as tc:
        with (
            tc.tile_pool(name="pool", bufs=1) as pool,
            tc.tile_pool(name="psum", bufs=1, space="PSUM") as psum,
        ):
            ones_col = pool.tile([P, 1], f32)
            nc.vector.memset(ones_col[:], 1.0)
            # value M_MUT: folds the *M of the closed form into the
            # K=1 broadcast matmul bc = m_row.T @ r
            m_row = pool.tile([1, W], f32)
            nc.vector.memset(m_row[:], float(M_MUT))

            # warm both ACT tables (Square, Sqrt) while DMAs run
            warm = pool.tile([1, 1], f32)
            nc.scalar.activation(warm[:], ones_col[0:1, 0:1], AF.Square)
            nc.scalar.activation(warm[:], ones_col[0:1, 0:1], AF.Sqrt)

            # f blocks on the HWDGE (sync) queue; column tensors on the
            # SWDGE (gpsimd) queue so they don't delay the stats backbone.
            fks = []
            for k in range(KB):
                fk = pool.tile([P, D], f16, name=f"fk{k}", tag="fk", bufs=KB)
                nc.sync.dma_start(fk[:], f_blk[k])
                fks.append(fk)
            v = pool.tile([P, KB * W], f32)
            a = pool.tile([P, KB * W], f16)
            fc = pool.tile([P, KB * W], f32)
            nc.gpsimd.dma_start(v[:], v_d[:])
            nc.gpsimd.dma_start(a[:], a_d[:])
            nc.gpsimd.dma_start(fc[:], fc_d[:])

            sumsq = pool.tile([P, KB], f32)
            sumf = pool.tile([P, KB], f32)
            t1 = pool.tile([P, KB], f32)
            u = pool.tile([P, KB], f32)
            s = pool.tile([P, KB], f32)
            t = pool.tile([P, KB * W], f32)
            invw = pool.tile([P, KB * W], f32)
            numer = pool.tile([P, KB * W], f32)
            y = pool.tile([P, KB * W], f32)
            z = pool.tile([P, KB * W], f32)
            part = psum.tile([1, W], f32)

            for k in range(KB):
                c0 = k * W
                ck = slice(c0, c0 + W)
                kk = slice(k, k + 1)
                # --- row stats for block k ---
                sqk = pool.tile([P, D], f32, name=f"sq{k}", tag="sq", bufs=2)
                nc.scalar.activation(
                    sqk[:], fks[k][:], AF.Square, accum_out=sumsq[:, kk]
                )
                nc.vector.reduce_sum(
                    out=sumf[:, kk], in_=fks[k][:], axis=mybir.AxisListType.X
                )
                nc.vector.tensor_mul(t1[:, kk], sumf[:, kk], sumf[:, kk])
                # u = sumsq - sumf^2/D = (D-1)*std^2
                nc.vector.tensor_scalar(
                    out=u[:, kk],
                    in0=t1[:, kk],
                    scalar1=-1.0 / D,
                    scalar2=sumsq[:, kk],
                    op0=OP.mult,
                    op1=OP.add,
                )
                nc.scalar.activation(
                    s[:, kk], u[:, kk], AF.Sqrt, scale=1.0 / (D - 1)
                )
                # --- column phase for block k ---
                # t = u*Vp (= var+eps, folded) ; invw = 1/t
                nc.gpsimd.tensor_scalar_mul(t[:, ck], v[:, ck], u[:, kk])
                nc.vector.reciprocal_approx_fast(invw[:, ck], t[:, ck])
                # colsum accumulates over blocks in PSUM
                nc.tensor.matmul(
                    part[:],
                    ones_col[:],
                    invw[:, ck],
                    start=(k == 0),
                    stop=(k == KB - 1),
                )
                # numer = A*s + f ; y = numer*invw
                nc.vector.scalar_tensor_tensor(
                    out=numer[:, ck],
                    in0=a[:, ck],
                    scalar=s[:, kk],
                    in1=fc[:, ck],
                    op0=OP.mult,
                    op1=OP.add,
                )
                nc.gpsimd.tensor_mul(y[:, ck], numer[:, ck], invw[:, ck])

            cs = pool.tile([1, W], f32)
            nc.vector.tensor_copy(cs[:], part[:])
            r = pool.tile([1, W], f32)
            nc.vector.reciprocal_approx_fast(r[:], cs[:])
            bc = psum.tile([P, W], f32)
            nc.tensor.matmul(bc[:], m_row[:], r[:])

            # z = y * (M/colsum) broadcast over blocks; DMA out in two halves
            half = KB // 2
            for g in range(2):
                cg = slice(g * half * W, (g + 1) * half * W)
                nc.vector.tensor_tensor(
                    out=z[:, cg].rearrange("p (k w) -> p k w", k=half),
                    in0=y[:, cg].rearrange("p (k w) -> p k w", k=half),
                    in1=bcast_w(bc[:], half),
                    op=OP.mult,
                )
                nc.sync.dma_start(z_d[0:P, cg], z[:, cg])

    nc.compile()
    _CACHE["nc"] = nc
    return nc


def _blockify(x):
    """[B, W] -> [P, KB*W]: row 128k+p lands at [p, k*W:(k+1)*W]."""
    Bx, Wx = x.shape
    return np.ascontiguousarray(
        x.reshape(KB, P, Wx).transpose(1, 0, 2).reshape(P, KB * Wx)
    )


def _unblockify(x):
    """[P, KB*W] -> [B, W] (inverse of _blockify)."""
    Px, KW = x.shape
    Wx = KW // KB
    return x.reshape(P, KB, Wx).transpose(1, 0, 2).reshape(KB * P, Wx)


def _in_maps(spatial, spectral):
    g = _gstats()
    halves = {"sp": (spatial, *g["sp"]), "spec": (spectral, *g["spec"])}
    maps = []
    for c in range(NCORES):
        half = "sp" if c < 4 else "spec"
        f_h, A_h, Vp_h = halves[half]
        ccol = (c % 4) * W
        maps.append(
            {
                "f_blk": np.ascontiguousarray(
                    f_h.reshape(KB, P, D).astype(np.float16)
                ),
                "fc": _blockify(f_h[:, ccol : ccol + W]),
                "ac": _blockify(A_h[:, ccol : ccol + W]),
                "vc": _blockify(Vp_h[:, ccol : ccol + W]),
            }
        )
    return maps


def run(spatial_features, spectral_features, trace=False, **kwargs):
    """Run the SPMD bass kernel; returns (Z [1024,1024] f32, BassKernelResults)."""
    from concourse.bass_utils import run_bass_kernel_spmd

    spatial = np.ascontiguousarray(np.asarray(spatial_features, dtype=np.float32))
    spectral = np.ascontiguousarray(np.asarray(spectral_features, dtype=np.float32))
    assert spatial.shape == (B, D) and spectral.shape == (B, D)

    nc = _build_bass()
    res = run_bass_kernel_spmd(
        nc,
        _in_maps(spatial, spectral),
        core_ids=list(range(NCORES)),
        trace=trace,
        **kwargs,
    )
    z = np.empty((B, 2 * D), dtype=np.float32)
    for c in range(NCORES):
        z[:, c * W : (c + 1) * W] = _unblockify(res.results[c]["z"])
    return z, res


def kernel(spatial_features, spectral_features):
    z, _ = run(spatial_features, spectral_features, trace=False)
    return z


# revision 12
# speedup vs baseline: 1.3482x; 1.3482x over previous
"""Trainium2 Bass kernel for nn_DMF_50130858279442.

Reference computation (M=100 Monte-Carlo mutations, fixed RNG key 42):
    std_h[b]   = std(f_h[b,:], ddof=1)                     (per half h)
    G_h        = mask * normal_draw            [M,B,D]     (input-INDEPENDENT)
    cand       = f_h[None] + G_h * std_h                   [M,B,D]
    var[b,d]   = var_m(cand) + eps  = std_h[b]^2 * var_m(G_h)[b,d] + eps
    w          = (1/var) / sum_b(1/var)
    Z[b,d]     = sum_m(cand*w) = w * M * (f_h[b,d] + std_h[b]*mean_m(G_h)[b,d])

Because the RNG key is fixed, A = mean_m(G) and V = var_m(G) are constants
([B,D] per half) precomputed once on host CPU. The device kernel computes the
closed form.

Sharding: over the 2D=1024 OUTPUT COLUMNS (128 per core) — cores 0-3 own the
spatial half's columns, 4-7 the spectral half's. The batch-axis normalization
sum is then core-local (no collective; an AllReduce here pays a ~47us
cross-core launch-skew barrier on this stack). Each core redundantly computes
the row-std of its own half from an fp16 copy (1 MiB DMA; stats error ~1e-4).

Per-core layout: the 1024 batch rows fold into KB=8 row blocks of P=128 (the
SBUF partition dim); block k sits at free offset k*W. Row b=128k+p is at
[partition p, block k]. The whole pipeline is per-block so everything hides
behind the serial ACT square backbone.

Numerics shortcuts (all ~<=1e-4 rel, vs typical 1e-2 tolerances):
  - fp16 f for row stats; fp16 A (enters as small correction s*A to f).
  - eps folded into the V constant: t = u*(V+eps)/(D-1) = var + eps*std^2.
  - reciprocal_approx_fast (~51 ULP) instead of DVE RECIPROCAL (5x faster).
"""

import numpy as np

P = 128          # SBUF partitions = rows per block
KB = 8           # row blocks (B = KB*P)
D = 512          # per-half feature dim
W = 128          # output columns per core
NCORES = 8
M_MUT = 100      # NUM_MUTATIONS
RATE = 0.2       # MUTATION_RATE
EPS = 1e-6
B = 1024

_CACHE = {}


def _gstats():
    """A = mean_m(mask*normal), Vp = (var_m(mask*normal)+eps)/(D-1), both halves.

    Computed once on host CPU with the reference's exact RNG (jax threefry,
    key 42). Vp folds both the 1/(D-1) of the unbiased row-variance and the
    reference's +eps, so the device computes var_cand+eps ~= u * Vp with
    u = sumsq - sum^2/D (= (D-1)*std^2).
    """
    if "gstats" in _CACHE:
        return _CACHE["gstats"]
    import jax
    import jax.numpy as jnp

    cpu = jax.devices("cpu")[0]
    out = {}
    with jax.default_device(cpu):
        rng = jax.random.key(42)
        km1, kn1, km2, kn2 = jax.random.split(rng, 4)
        for name, km, kn in (("sp", km1, kn1), ("spec", km2, kn2)):
            mask = np.asarray(
                jax.random.uniform(km, (M_MUT, B, D), dtype=jnp.float32) < RATE
            )
            noise = np.asarray(
                jax.random.normal(kn, (M_MUT, B, D), dtype=jnp.float32)
            )
            G = np.where(mask, noise, np.float32(0.0)).astype(np.float64)
            A = G.mean(axis=0)
            V = G.var(axis=0)  # ddof=0, matches jnp.var over mutations
            out[name] = (
                np.ascontiguousarray(A, dtype=np.float16),
                np.ascontiguousarray((V + EPS) / (D - 1), dtype=np.float32),
            )
    _CACHE["gstats"] = out
    return out


def _build_bass():
    """Per-core SPMD Tile kernel (identical program; per-core data differs)."""
    if "nc" in _CACHE:
        return _CACHE["nc"]
    import concourse.bacc as bacc
    import concourse.mybir as mybir
    from concourse import bass
    from concourse import tile

    f32 = mybir.dt.float32
    f16 = mybir.dt.float16
    AF = mybir.ActivationFunctionType
    OP = mybir.AluOpType

    nc = bacc.Bacc(
        "TRN2", target_bir_lowering=False, debug=False, num_devices=NCORES
    )

    # the core's FULL half in fp16, block-major [KB, P, D] (row stats only)
    f_blk = nc.dram_tensor("f_blk", [KB, P, D], f16, kind="ExternalInput")
    # column slices for this core's W output columns, block-concat [P, KB*W]
    fc_d = nc.dram_tensor("fc", [P, KB * W], f32, kind="ExternalInput")
    a_d = nc.dram_tensor("ac", [P, KB * W], f16, kind="ExternalInput")
    v_d = nc.dram_tensor("vc", [P, KB * W], f32, kind="ExternalInput")
    z_d = nc.dram_tensor("z", [P, KB * W], f32, kind="ExternalOutput")

    def bcast_w(ap, blocks=KB):
        # free-axis broadcast: [P, W] -> [P, (blocks), W] with step-0 mid dim
        return bass.AP(ap.tensor, ap.offset, [ap.ap[0], [0, blocks], ap.ap[1]])

    with tile.TileContext(nc) as tc:
        with (
            tc.tile_pool(name="pool", bufs=1) as pool,
            tc.tile_pool(name="psum", bufs=1, space="PSUM") as psum,
        ):
            ones_col = pool.tile([P, 1], f32)
            nc.vector.memset(ones_col[:], 1.0)
            # value M_MUT: folds the *M of the closed form into the
            # K=1 broadcast matmul bc = m_row.T @ r
            m_row = pool.tile([1, W], f32)
            nc.vector.memset(m_row[:], float(M_MUT))

            # warm both ACT tables (Square, Sqrt) while DMAs run
            warm = pool.tile([1, 1], f32)
            nc.scalar.activation(warm[:], ones_col[0:1, 0:1], AF.Square)
            nc.scalar.activation(warm[:], ones_col[0:1, 0:1], AF.Sqrt)

            # Per-queue DMA completions serialize (~2us receipt each), so use
            # few, chunked DMAs across all three DGE queues (sync/scalar HWDGE,
            # gpsimd SWDGE).
            fks = [
                pool.tile([P, D], f16, name=f"fk{k}", tag="fk", bufs=KB)
                for k in range(KB)
            ]

            def f_chunk(engine, k0, k1):
                dst = [fks[k][:] for k in range(k0, k1)]
                for k in range(k0, k1):
                    engine.dma_start(fks[k][:], f_blk[k])

            v = pool.tile([P, KB * W], f32)
            a = pool.tile([P, KB * W], f16)
            fc = pool.tile([P, KB * W], f32)
            nc.gpsimd.dma_start(v[:], v_d[:])
            f_chunk(nc.sync, 0, 3)
            f_chunk(nc.scalar, 3, 6)
            f_chunk(nc.gpsimd, 6, 8)
            nc.sync.dma_start(fc[:], fc_d[:])
            nc.scalar.dma_start(a[:], a_d[:])

            sumsq = pool.tile([P, KB], f32)
            sumf = pool.tile([P, KB], f32)
            t1 = pool.tile([P, KB], f32)
            u = pool.tile([P, KB], f32)
            s = pool.tile([P, KB], f32)
            t = pool.tile([P, KB * W], f32)
            invw = pool.tile([P, KB * W], f32)
            numer = pool.tile([P, KB * W], f32)
            y = pool.tile([P, KB * W], f32)
            z = pool.tile([P, KB * W], f32)
            part = psum.tile([1, W], f32)

            for k in range(KB):
                c0 = k * W
                ck = slice(c0, c0 + W)
                kk = slice(k, k + 1)
                # --- row stats for block k ---
                sqk = pool.tile([P, D], f32, name=f"sq{k}", tag="sq", bufs=2)
                nc.scalar.activation(
                    sqk[:], fks[k][:], AF.Square, accum_out=sumsq[:, kk]
                )
                nc.vector.reduce_sum(
                    out=sumf[:, kk], in_=fks[k][:], axis=mybir.AxisListType.X
                )
                nc.vector.tensor_mul(t1[:, kk], sumf[:, kk], sumf[:, kk])
                # u = sumsq - sumf^2/D = (D-1)*std^2
                nc.vector.tensor_scalar(
                    out=u[:, kk],
                    in0=t1[:, kk],
                    scalar1=-1.0 / D,
                    scalar2=sumsq[:, kk],
                    op0=OP.mult,
                    op1=OP.add,
                )
                nc.scalar.activation(
                    s[:, kk], u[:, kk], AF.Sqrt, scale=1.0 / (D - 1)
                )
                # --- column phase for block k ---
                # t = u*Vp (= var+eps, folded) ; invw = 1/t
                nc.vector.tensor_scalar_mul(t[:, ck], v[:, ck], u[:, kk])
                nc.vector.reciprocal_approx_fast(invw[:, ck], t[:, ck])
                # colsum accumulates over blocks in PSUM
                nc.tensor.matmul(
                    part[:],
                    ones_col[:],
                    invw[:, ck],
                    start=(k == 0),
                    stop=(k == KB - 1),
                )
                # numer = A*s + f ; y = numer*invw
                nc.vector.scalar_tensor_tensor(
                    out=numer[:, ck],
                    in0=a[:, ck],
                    scalar=s[:, kk],
                    in1=fc[:, ck],
                    op0=OP.mult,
                    op1=OP.add,
                )
                nc.gpsimd.tensor_mul(y[:, ck], numer[:, ck], invw[:, ck])

            cs = pool.tile([1, W], f32)
            nc.vector.tensor_copy(cs[:], part[:])
            r = pool.tile([1, W], f32)
            nc.vector.reciprocal_approx_fast(r[:], cs[:])
            bc = psum.tile([P, W], f32)
            nc.tensor.matmul(bc[:], m_row[:], r[:])

            # z = y * (M/colsum) broadcast over blocks; DMA out in two halves
            half = KB // 2
            for g in range(2):
                cg = slice(g * half * W, (g + 1) * half * W)
                nc.vector.tensor_tensor(
                    out=z[:, cg].rearrange("p (k w) -> p k w", k=half),
                    in0=y[:, cg].rearrange("p (k w) -> p k w", k=half),
                    in1=bcast_w(bc[:], half),
                    op=OP.mult,
                )
                nc.sync.dma_start(z_d[0:P, cg], z[:, cg])

    nc.compile()
    _CACHE["nc"] = nc
    return nc


def _blockify(x):
    """[B, W] -> [P, KB*W]: row 128k+p lands at [p, k*W:(k+1)*W]."""
    Bx, Wx = x.shape
    return np.ascontiguousarray(
        x.reshape(KB, P, Wx).transpose(1, 0, 2).reshape(P, KB * Wx)
    )


def _unblockify(x):
    """[P, KB*W] -> [B, W] (inverse of _blockify)."""
    Px, KW = x.shape
    Wx = KW // KB
    return x.reshape(P, KB, Wx).transpose(1, 0, 2).reshape(KB * P, Wx)


def _in_maps(spatial, spectral):
    g = _gstats()
    halves = {"sp": (spatial, *g["sp"]), "spec": (spectral, *g["spec"])}
    maps = []
    for c in range(NCORES):
        half = "sp" if c < 4 else "spec"
        f_h, A_h, Vp_h = halves[half]
        ccol = (c % 4) * W
        maps.append(
            {
                "f_blk": np.ascontiguousarray(
                    f_h.reshape(KB, P, D).astype(np.float16)
                ),
                "fc": _blockify(f_h[:, ccol : ccol + W]),
                "ac": _blockify(A_h[:, ccol : ccol + W]),
                "vc": _blockify(Vp_h[:, ccol : ccol + W]),
            }
        )
    return maps


def run(spatial_features, spectral_features, trace=False, **kwargs):
    """Run the SPMD bass kernel; returns (Z [1024,1024] f32, BassKernelResults)."""
    from concourse.bass_utils import run_bass_kernel_spmd

    spatial = np.ascontiguousarray(np.asarray(spatial_features, dtype=np.float32))
    spectral = np.ascontiguousarray(np.asarray(spectral_features, dtype=np.float32))
    assert spatial.shape == (B, D) and spectral.shape == (B, D)

    nc = _build_bass()
    res = run_bass_kernel_spmd(
        nc,
        _in_maps(spatial, spectral),
        core_ids=list(range(NCORES)),
        trace=trace,
        **kwargs,
    )
    z = np.empty((B, 2 * D), dtype=np.float32)
    for c in range(NCORES):
        z[:, c * W : (c + 1) * W] = _unblockify(res.results[c]["z"])
    return z, res


def kernel(spatial_features, spectral_features):
    z, _ = run(spatial_features, spectral_features, trace=False)
    return z
